# revision 42
# baseline (speedup 1.0000x reference)
"""Trainium2 Bass kernel for nn_LongformerPersonalizedClsHead (MoE routing head).

Reference computation (B=256, S=512, H=768, U=100, L=2):
    x  = hidden_states[:, 0, :]                      # [B, H]  (CLS token only)
    z  = sum_u mask[b,u] * (x @ dense_W[u]) + mask @ dense_b
    h  = tanh(z)
    out= sum_u mask[b,u] * (h @ out_proj_W[u]) + mask @ out_proj_b   # [B, L]

Strategy: expert-parallel layer 1 over the 8 NeuronCores of one trn2 chip
(13 expert slots/core, U padded 100->104). Each core streams its 13 [768,768]
expert matrices from HBM in bf16 (~15 MB/core — the memory roofline), computes
per-expert y_u = x @ W_u on the tensor engine, and folds it into a partial
z += mask[:,u] * y_u with one fused scalar_tensor_tensor per tile on DVE
(accumulated in bf16, k-columns permuted per-core into "local slice" order).

The cross-core reduction of z avoids the collectives stack entirely (the
cost of a single ReduceScatter exceeds the whole reduction here): a direct
SBUF-to-SBUF remote-DMA XOR mesh. Local slice k (96 k-columns) is the global
slice owned by the core at physical XOR-distance k; when the stream ends the
core fires 7 remote_dma_broadcast sends (relative dests, drid=0, dtpb=k, one
SWDGE trigger for all of them — partial triggers proved unreliable on HW)
that land in the peers' receive slots and bump a per-slice remote semaphore.
Each core sums its own slice plus the 7 received contributions as they
arrive (Pool-engine adds gated per-slot), applies tanh, and runs its 96-row
share of the output projection against all 100 users; the host sums the 8
[256, 2] partials.

The physical XOR topology (logical->physical TPB permutation) is discovered
once at runtime by a tiny probe kernel; a hardcoded trn2 fallback is used if
the probe fails.
"""
import numpy as np

B, S, H, U, L = 256, 512, 768, 100, 2
N_CORES = 8
UPC = 13            # expert slots per core (8*13 = 104 >= 100, zero-padded)
UPAD = N_CORES * UPC
NB = B // 128       # 2 batch tiles
NH = H // 128       # 6 contraction chunks
SLW = H // N_CORES  # 96-column slice owned by each core
# two column pieces, slice-aligned, both with >=512B DMA descriptors in bf16:
PIECES = ((0, 5), (5, 8))   # piece -> (first slice, end slice)
L2W = L * U         # 200 columns of the layer-2 moving operand

_RUNNER = None
_PERM = None


def _build_nc():
    import concourse.bacc as bacc
    import concourse.mybir as mybir
    import concourse.tile as tile
    from concourse.masks import make_identity

    f32 = mybir.dt.float32
    f32r = mybir.dt.float32r
    bf16 = mybir.dt.bfloat16

    nc = bacc.Bacc("TRN2", target_bir_lowering=False)

    xT = nc.dram_tensor("xT", [H, B], bf16, kind="ExternalInput")
    w = nc.dram_tensor("w", [UPC, H, H], bf16, kind="ExternalInput")
    mask = nc.dram_tensor("mask", [B, UPC], f32, kind="ExternalInput")
    maskT = nc.dram_tensor("maskT", [UPC, B], f32r, kind="ExternalInput")
    db = nc.dram_tensor("db", [UPC, H], f32r, kind="ExternalInput")
    woT = nc.dram_tensor("woT", [SLW, L2W], bf16, kind="ExternalInput")
    bo = nc.dram_tensor("bo", [1, L2W], f32, kind="ExternalInput")
    mrep = nc.dram_tensor("mrep", [B, L2W], f32, kind="ExternalInput")
    o = nc.dram_tensor("o", [B, L], f32, kind="ExternalOutput")

    rsems = [nc.alloc_semaphore(f"rsem{k}") for k in range(1, N_CORES)]
    lsem = nc.alloc_semaphore("lsem")
    gsem = nc.alloc_semaphore("gsem")
    dsem = nc.alloc_semaphore("dsem")
    waits_to_patch = []
    trig_holder = []

    with tile.TileContext(nc) as tc:
        with (
            tc.tile_pool(name="const", bufs=1) as cpool,
            tc.tile_pool(name="wpool", bufs=6) as wpool,
            tc.tile_pool(name="py", bufs=4, space="PSUM") as psum_y,
            tc.tile_pool(name="pmisc", bufs=2, space="PSUM") as psum_m,
            tc.tile_pool(name="pq", bufs=2, space="PSUM") as psum_q,
        ):
            # --- resident inputs (xT + first weights first: they gate the
            # stream; layer-2 inputs are deferred until the stream is going) ---
            xT_sb = cpool.tile([128, NH, B], bf16, tag="xT")
            nc.sync.dma_start(xT_sb[:], xT.rearrange("(p c) b -> p c b", p=128))
            s0, s1 = PIECES[0]
            w_first = wpool.tile([128, NH, (s1 - s0) * SLW], bf16, tag="w")
            nc.sync.dma_start(
                w_first[:],
                w[0, :, s0 * SLW:s1 * SLW].rearrange("(p c) k -> p c k", p=128))
            maskT_sb = cpool.tile([UPC, B], f32r, tag="maskT")
            nc.sync.dma_start(maskT_sb[:], maskT[:])
            db_sb = cpool.tile([UPC, H], f32r, tag="db")
            nc.sync.dma_start(db_sb[:], db[:])
            mask_sb = cpool.tile([128, NB, UPC], f32, tag="mask")
            nc.sync.dma_start(mask_sb[:], mask.rearrange("(nb p) u -> p nb u", p=128))
            ones = cpool.tile([1, 512], f32, tag="ones")
            nc.vector.memset(ones[:], 1.0)

            # p-state warmup: keep PE busy continuously from t~1us until the
            # first expert weights land, so the 2.4 GHz clock is reached
            # before the real stream starts (junk matmuls, never read)
            warm = psum_y.tile([128, 480], f32, tag="y")
            for cols in (480, 480, 384):
                nc.tensor.matmul(warm[:, :cols], ones[:, :128], ones[:, :cols],
                                 start=True, stop=True, skip_group_check=True)

            # z accumulator, k-columns grouped by local slice (bf16)
            z2_sb = cpool.tile([128, N_CORES, NB, SLW], bf16, tag="z2")
            # peer contributions land here: slot k-1 <- peer at XOR-distance k
            recv_sb = cpool.tile([128, N_CORES - 1, NB, SLW], bf16, tag="recv")

            # descriptor generation for all 7 sends runs up front on the
            # idle Pool engine (descgen reads no data); the trigger's wait is
            # patched after scheduling to the final STT's DVE engine tick,
            # which on the in-order DVE queue covers every z2 write.
            for k in range(1, N_CORES):
                nc.gpsimd.remote_dma_broadcast(
                    recv_sb[:, k - 1, :, :], z2_sb[:, k, :, :],
                    remote_sem=rsems[k - 1], local_sem=lsem,
                    rdests=[(0, k) if j == k else None
                            for j in range(N_CORES)],
                )


            # --- expert stream: z += mask[:,u] * (x @ W_u), piece by piece;
            # after a piece completes its slices are sent to their owners ---
            for pi, (s0, s1) in enumerate(PIECES):
                pw = (s1 - s0) * SLW
                for u in range(UPC):
                    if pi == 0 and u == 0:
                        w_sb = w_first
                    else:
                        w_sb = wpool.tile([128, NH, pw], bf16, tag="w")
                        nc.sync.dma_start(
                            w_sb[:],
                            w[u, :, s0 * SLW:s1 * SLW].rearrange(
                                "(p c) k -> p c k", p=128),
                        )
                    for b in range(NB):
                        acc = psum_y.tile([128, 480], f32, tag="y")
                        for hc in range(NH):
                            nc.tensor.matmul(
                                acc[:, :pw],
                                xT_sb[:, hc, b * 128:(b + 1) * 128],
                                w_sb[:, hc, :],
                                start=(hc == 0), stop=(hc == NH - 1),
                            )
                        if pi == 0 and u == 0:
                            # layer-1 bias partial seeds z, both pieces
                            # (placed here so the tiny seed matmuls don't
                            # delay the stream start)
                            for (ss0, ss1) in PIECES:
                                sw = (ss1 - ss0) * SLW
                                sacc = psum_y.tile([128, 480], f32, tag="y")
                                nc.tensor.matmul(
                                    sacc[:, :sw],
                                    maskT_sb[:, b * 128:(b + 1) * 128],
                                    db_sb[:, ss0 * SLW:ss1 * SLW],
                                    start=True, stop=True,
                                )
                                nc.scalar.copy(
                                    z2_sb[:, ss0:ss1, b, :],
                                    sacc[:, :sw].rearrange(
                                        "p (s k) -> p s k", k=SLW),
                                )
                        nc.vector.scalar_tensor_tensor(
                            z2_sb[:, s0:s1, b, :],
                            acc[:, :pw].rearrange("p (s k) -> p s k", k=SLW),
                            mask_sb[:, b, u:u + 1],
                            z2_sb[:, s0:s1, b, :],
                            mybir.AluOpType.mult, mybir.AluOpType.add,
                        )
                # one trigger fires all 7 sends (the SWDGE trigger path
                # misbehaves on overlapping triggers); gsem orders the reduce
                # adds after the trigger in the scheduler's in-order Pool
                # queue. The dsem placeholder wait is rewritten after
                # scheduling to the final STT's DVE-tick condition.
                if pi == len(PIECES) - 1:
                    trig = nc.gpsimd.trigger_dma(count=None)._wait_ge(dsem, 0)
                    trig.then_inc(gsem)
                    trig_holder.append(trig)

            # --- reduce: own slice + 7 peer contributions (f32 accumulator).
            # The first 4 arrive with piece 0, the rest with piece 1; the
            # remote-sem waits are patched in after scheduling (the Tile
            # scheduler's single-core sim cannot see remote increments). ---
            ps_sb = cpool.tile([128, NB, SLW], f32, tag="ps")
            i0 = nc.gpsimd.tensor_add(
                ps_sb[:], z2_sb[:, 0, :, :], recv_sb[:, 0, :, :]
            )._wait_ge(gsem, 1)
            waits_to_patch.append((i0, rsems[0], 2))
            for j in range(1, N_CORES - 1):
                ij = nc.gpsimd.tensor_add(
                    ps_sb[:], ps_sb[:], recv_sb[:, j, :, :]
                )._wait_ge(gsem, 1)
                waits_to_patch.append((ij, rsems[j], 2))

            # --- deferred layer-2 inputs ---
            woT_sb = cpool.tile([SLW, L2W], bf16, tag="woT")
            nc.sync.dma_start(woT_sb[:], woT[:])
            bo_sb = cpool.tile([1, L2W], f32, tag="bo")
            nc.sync.dma_start(bo_sb[:], bo[:])
            mrep_sb = cpool.tile([128, NB, L2W], f32, tag="mrep")
            nc.sync.dma_start(mrep_sb[:], mrep.rearrange("(nb p) l -> p nb l", p=128))
            ident = cpool.tile([128, 128], f32, tag="ident")
            make_identity(nc, ident[:])

            # --- tail: transpose + tanh the owned slice, tiny layer 2 ---
            o_sb = cpool.tile([128, NB, L], f32, tag="o")
            for b in range(NB):
                tp = psum_m.tile([SLW, 128], f32, tag="tp")
                nc.tensor.transpose(tp[:], ps_sb[:, b, :], ident[:])
                hT_sb = cpool.tile([SLW, 128], bf16, tag=f"hT{b}")
                nc.scalar.activation(
                    hT_sb[:], tp[:], mybir.ActivationFunctionType.Tanh)
                q = psum_q.tile([128, L2W], f32, tag="q")
                nc.tensor.matmul(q[:], ones[:, :128], bo_sb[:],
                                 start=True, stop=False)
                nc.tensor.matmul(q[:], hT_sb[:], woT_sb[:],
                                 start=False, stop=True)
                p2 = cpool.tile([128, L2W], f32, tag=f"p2{b}")
                nc.vector.tensor_mul(p2[:], q[:], mrep_sb[:, b, :])
                nc.vector.reduce_sum(
                    o_sb[:, b, :],
                    p2[:].rearrange("p (l u) -> p l u", u=U),
                    axis=mybir.AxisListType.X,
                )
            nc.sync.dma_start(o.rearrange("(nb p) l -> p nb l", p=128), o_sb[:])

    # Gate the trigger on the final STT's completion: find the DVE engine
    # tick the last STT updates and the cumulative count at that point, and
    # rewrite the trigger's dsem placeholder wait to it.
    import concourse.bass_isa as bass_isa
    insts = [i for blk in nc.m.functions[0].blocks for i in blk.instructions]
    stts = [i for i in insts if isinstance(i, mybir.InstTensorScalarPtr)]
    last_stt = stts[-1]
    upd = [u for u in last_stt.sync_info.on_update
           if u.ant_name and u.ant_name.startswith("DVE")]
    assert upd, "last STT carries no DVE tick update"
    tick_id, tick_name = upd[0].id, upd[0].ant_name
    count = 0
    for i in insts:
        si = getattr(i, "sync_info", None)
        if si is None:
            continue
        for u in si.on_update:
            if u.id == tick_id:
                count += u.update_value if u.update_value is not None else 1
        if i is last_stt:
            break
    for sw in trig_holder[0].ins.sync_info.on_wait:
        if sw.ant_name == "dsem":
            sw.id = tick_id
            sw.ant_name = tick_name
            sw.wait_value = count

    # Rewrite the adds' schedule-ordering gsem waits into the true per-slot
    # arrival waits (2 per send on that slice's remote sem).
    for inst, sem, val in waits_to_patch:
        for sw in inst.ins.sync_info.on_wait:
            if sw.ant_name == "gsem":
                sw.id = sem.num
                sw.ant_name = sem.name
                sw.wait_value = val
    nc.finalize()
    nc._rsems = {sem.name: sem for sem in rsems}
    return nc


def _sim_cost_model():
    """Cost model for TimelineSim that fills the stock model's no_exec gap for
    the remote-DMA mesh: remote_dma_broadcast preps are recorded at visit
    time, and the matching InstTriggerDma gets an extra DMA_ENGINES track
    carrying the transfer delays plus the remote-sem updates. Each update is
    applied to the LOCAL remote-sem — the single-core timeline proxy for a
    symmetric SPMD mesh (my send to a peer completes when the peer's
    symmetric send to me lands here)."""
    import concourse.mybir as mybir
    import concourse.bass_isa as bass_isa
    from concourse.cost_model import (
        Delay, DeviceAcquire, DeviceFree, InstructionCostModel,
        NonEngineDevice, SemUpdate,
    )
    from concourse.hw_specs import TRN2Spec

    SEQ = mybir.EngineType.Pool

    class RdmaLoopback(InstructionCostModel):
        rsems = None

        def __init__(self, hw_spec):
            super().__init__(hw_spec)
            self._pending = {}

        def visit(self, instruction, sim):
            tracks = self._state.visit(instruction, sim)
            if isinstance(instruction, bass_isa.InstRemoteDMABroadcastDescs):
                n_dests = len(instruction.dests)
                n_real = sum(1 for d in instruction.dests if d >= 0)
                bw = TRN2Spec.RDMA_D2D_BANDWIDTH_BYTES_PER_NS_PER_ENGINE * 16.0
                self._pending.setdefault(instruction.queue_num, []).append((
                    n_dests * instruction.free_dim_bytes * 128.0 / bw,
                    instruction.remote_sem_name,
                    n_real * (16 // max(n_dests, 1)),
                ))
            elif (isinstance(instruction, bass_isa.InstTriggerDma)
                  and self._pending.get(instruction.queue_num)
                  and self.rsems is not None):
                fire = self._pending.pop(instruction.queue_num)
                extra = [
                    DeviceAcquire(_engine_device(SEQ)),
                    Delay(1.0),
                    DeviceFree(_engine_device(SEQ)),
                    DeviceAcquire(NonEngineDevice.DMA_ENGINES),
                ]
                from concourse.bass import create_sync_update
                for t_ns, sem_name, inc in fire:
                    extra.append(Delay(t_ns))
                    extra.append(SemUpdate(
                        create_sync_update(self.rsems[sem_name], inc)))
                extra.append(Delay(TRN2Spec.RDMA_D2D_ACK_LATENCY_NS))
                extra.append(DeviceFree(NonEngineDevice.DMA_ENGINES))
                tracks = list(tracks) + [extra]
            return tracks

    def _engine_device(engine):
        # Device = tuple[EngineType, EngComponent] on the Python side
        from concourse.hw_specs import EngComponent
        return (engine, EngComponent.SEQ)

    return RdmaLoopback(TRN2Spec)


def sim_ns():
    """Cost-model timeline estimate for this kernel (ns)."""
    from concourse.timeline_sim import TimelineSim
    nc = _build_nc()
    cm = _sim_cost_model()
    cm.rsems = nc._rsems
    return TimelineSim(nc, cost_model=cm).simulate()


# ---------------------------------------------------------------------------
# Topology probe: seen[c][k] = logical id of the core at physical XOR
# distance k from core c. Row c is also the slice permutation for core c.
# ---------------------------------------------------------------------------

def _fallback_perm():
    # trn2 logical->physical TPB map: dies pair as P(c) = c ^ (2 if c&4)
    P = [c ^ 2 if c & 4 else c for c in range(N_CORES)]
    return np.array([[P[P[c] ^ k] for k in range(N_CORES)]
                     for c in range(N_CORES)], dtype=np.int64)


def _probe_nc():
    import concourse.bacc as bacc
    import concourse.mybir as mybir
    import concourse.tile as tile

    f32 = mybir.dt.float32
    W = 8
    nc = bacc.Bacc("TRN2", target_bir_lowering=False)
    tag = nc.dram_tensor("tag", [128, W], f32, kind="ExternalInput")
    seen = nc.dram_tensor("seen", [1, N_CORES * W], f32, kind="ExternalOutput")
    rsem = nc.alloc_semaphore("rsem")
    lsem = nc.alloc_semaphore("lsem")
    with tile.TileContext(nc) as tc:
        with tc.tile_pool(name="p", bufs=1) as pool:
            tag_sb = pool.tile([128, W], f32, tag="tag")
            nc.sync.dma_start(tag_sb[:], tag[:])
            recv_sb = pool.tile([128, N_CORES, W], f32, tag="recv")
            nc.vector.memset(recv_sb[:], -1.0)
            for k in range(N_CORES):
                nc.gpsimd.remote_dma_broadcast(
                    recv_sb[:, k, :], tag_sb[:],
                    remote_sem=rsem, local_sem=lsem,
                    rdests=[(0, k) if j == k else None for j in range(N_CORES)],
                )
            nc.gpsimd.trigger_dma(count=None)
            out_dma = nc.sync.dma_start(
                seen.rearrange("o (k w) -> o k w", k=N_CORES),
                recv_sb[0:1, :, :],
            )._wait_ge(rsem, 0)
    for sw in out_dma.ins.sync_info.on_wait:
        if sw.ant_name == "rsem":
            sw.wait_value = 2 * N_CORES
    nc.finalize()
    return nc


def _discover_perm():
    global _PERM
    if _PERM is not None:
        return _PERM
    try:
        runner = _SpmdRunner(_probe_nc(), N_CORES)
        in_maps = [{"tag": np.full((128, 8), float(c), np.float32)}
                   for c in range(N_CORES)]
        results = runner.run(in_maps)
        perm = np.zeros((N_CORES, N_CORES), dtype=np.int64)
        for c in range(N_CORES):
            row = results[c]["seen"].reshape(N_CORES, 8)[:, 0]
            perm[c] = row.astype(np.int64)
        valid = all(sorted(perm[c].tolist()) == list(range(N_CORES))
                    and perm[c][0] == c for c in range(N_CORES))
        _PERM = perm if valid else _fallback_perm()
    except Exception:
        _PERM = _fallback_perm()
    return _PERM


class _SpmdRunner:
    """Cached PJRT SPMD runner (mirrors concourse.bass2jax.run_bass_via_pjrt,
    but keeps the jitted callable alive so repeat calls don't re-trace)."""

    def __init__(self, nc, n_cores):
        import jax
        import concourse.mybir as mybir
        from concourse.bass2jax import (
            _bass_exec_p, install_neuronx_cc_hook, partition_id_tensor,
        )
        from jax.sharding import Mesh, PartitionSpec, NamedSharding
        try:
            from jax.experimental.shard_map import shard_map
        except ImportError:
            from jax.shard_map import shard_map

        install_neuronx_cc_hook()
        self.jax = jax
        self.nc = nc
        self.n_cores = n_cores

        in_names, out_names, out_avals, zero_outs = [], [], [], []
        partition_name = nc.partition_id_tensor.name if nc.partition_id_tensor else None
        dbg_name = None
        if nc.dbg_addr is not None:
            assert not nc.dbg_callbacks
            dbg_name = nc.dbg_addr.name
        for alloc in nc.m.functions[0].allocations:
            if not isinstance(alloc, mybir.MemoryLocationSet):
                continue
            name = alloc.memorylocations[0].name
            if alloc.kind == "ExternalInput":
                if name not in (partition_name, dbg_name):
                    in_names.append(name)
            elif alloc.kind == "ExternalOutput":
                out_names.append(name)
                shape = tuple(alloc.tensor_shape)
                dtype = mybir.dt.np(alloc.dtype)
                out_avals.append(jax.core.ShapedArray(shape, dtype))
                zero_outs.append(np.zeros(shape, dtype))

        self.in_names = list(in_names)
        self.out_names = list(out_names)
        self.zero_outs = zero_outs

        n_params = len(in_names)
        bound_names = list(in_names) + list(out_names)
        if dbg_name is not None:
            bound_names.append(dbg_name)
        if partition_name is not None:
            bound_names.append(partition_name)

        def _body(*args):
            operands = list(args)
            if dbg_name is not None:
                operands.append(jax.numpy.zeros((1, 2), jax.numpy.uint32))
            if partition_name is not None:
                operands.append(partition_id_tensor())
            outs = _bass_exec_p.bind(
                *operands,
                out_avals=tuple(out_avals),
                in_names=tuple(bound_names),
                out_names=tuple(self.out_names),
                lowering_input_output_aliases=(),
                sim_require_finite=True,
                sim_require_nnan=True,
                nc=nc,
            )
            return tuple(outs)

        devices = jax.devices()[:n_cores]
        assert len(devices) == n_cores, f"need {n_cores} cores, have {len(devices)}"
        self.mesh = Mesh(np.asarray(devices), ("core",))
        self.spec = PartitionSpec("core")
        self.sharding = NamedSharding(self.mesh, self.spec)
        n_args = n_params + len(out_names)
        self._jit = jax.jit(
            shard_map(
                _body,
                mesh=self.mesh,
                in_specs=(self.spec,) * n_args,
                out_specs=(self.spec,) * len(out_names),
                check_rep=False,
            ),
            keep_unused=True,
        )

    def put(self, in_maps):
        args = []
        for name in self.in_names:
            arrs = [np.asarray(in_maps[c][name]) for c in range(self.n_cores)]
            args.append(np.concatenate(arrs, axis=0))
        for z in self.zero_outs:
            args.append(np.concatenate([z] * self.n_cores, axis=0))
        return [self.jax.device_put(a, self.sharding) for a in args]

    def run_device(self, device_args):
        return self._jit(*device_args)

    def run(self, in_maps):
        outs = self._jit(*self.put(in_maps))
        np_outs = [np.asarray(o) for o in outs]
        results = []
        for c in range(self.n_cores):
            d = {}
            for i, name in enumerate(self.out_names):
                full = np_outs[i]
                per = full.shape[0] // self.n_cores
                d[name] = full[c * per:(c + 1) * per]
            results.append(d)
        return results


def _get_runner():
    global _RUNNER
    if _RUNNER is None:
        _RUNNER = _SpmdRunner(_build_nc(), N_CORES)
    return _RUNNER


def _prep_in_maps(hidden_states, user_mask, dense_W, dense_b, out_proj_W, out_proj_b):
    import ml_dtypes
    bf16 = ml_dtypes.bfloat16

    perm = _discover_perm()

    x = np.ascontiguousarray(hidden_states[:, 0, :], dtype=np.float32)   # [B, H]
    xT = np.ascontiguousarray(x.T).astype(bf16)                          # [H, B]

    # pad experts to 104
    pad = UPAD - U
    mask_p = np.concatenate([user_mask, np.zeros((B, pad), np.float32)], axis=1)
    w_p = np.concatenate([dense_W, np.zeros((pad, H, H), np.float32)], axis=0)
    db_p = np.concatenate([dense_b, np.zeros((pad, H), np.float32)], axis=0)

    # layer 2: every core gets its own 96-row slice of the full user bank,
    # the full mask (repeated for both logits), and bias/8 (host sums cores)
    woT_full = np.ascontiguousarray(
        out_proj_W.transpose(1, 2, 0).reshape(H, L2W)).astype(bf16)      # [H,(l,u)]
    bo8 = np.ascontiguousarray(out_proj_b.T.reshape(1, L2W)) / N_CORES   # [1,(l,u)]
    mrep = np.ascontiguousarray(
        np.concatenate([user_mask, user_mask], axis=1), dtype=np.float32)

    in_maps = []
    for c in range(N_CORES):
        sl = slice(c * UPC, (c + 1) * UPC)
        m = np.ascontiguousarray(mask_p[:, sl])                 # [B, UPC]
        # local slice k holds the global slice owned by the peer at physical
        # XOR distance k (perm[c][k]); k-columns of W/db permuted to match
        cols = np.concatenate(
            [np.arange(g * SLW, (g + 1) * SLW) for g in perm[c]])
        in_maps.append({
            "xT": xT,
            "w": np.ascontiguousarray(w_p[sl][:, :, cols]).astype(bf16),
            "mask": m,
            "maskT": np.ascontiguousarray(m.T),
            "db": np.ascontiguousarray(db_p[sl][:, cols]),
            "woT": np.ascontiguousarray(woT_full[c * SLW:(c + 1) * SLW]),
            "bo": bo8.astype(np.float32),
            "mrep": mrep,
        })
    return in_maps


def kernel(hidden_states, user_mask, dense_W, dense_b, out_proj_W, out_proj_b):
    hidden_states = np.asarray(hidden_states, dtype=np.float32)
    user_mask = np.asarray(user_mask, dtype=np.float32)
    dense_W = np.asarray(dense_W, dtype=np.float32)
    dense_b = np.asarray(dense_b, dtype=np.float32)
    out_proj_W = np.asarray(out_proj_W, dtype=np.float32)
    out_proj_b = np.asarray(out_proj_b, dtype=np.float32)

    runner = _get_runner()
    in_maps = _prep_in_maps(hidden_states, user_mask, dense_W, dense_b,
                            out_proj_W, out_proj_b)
    results = runner.run(in_maps)
    out = np.zeros((B, L), np.float32)
    for c in range(N_CORES):
        out += results[c]["o"]
    return np.ascontiguousarray(out, dtype=np.float32)


# revision 45
# speedup vs baseline: 1.0824x; 1.0824x over previous
"""Trainium2 Bass kernel for nn_LongformerPersonalizedClsHead (MoE routing head).

Reference computation (B=256, S=512, H=768, U=100, L=2):
    x  = hidden_states[:, 0, :]                      # [B, H]  (CLS token only)
    z  = sum_u mask[b,u] * (x @ dense_W[u]) + mask @ dense_b
    h  = tanh(z)
    out= sum_u mask[b,u] * (h @ out_proj_W[u]) + mask @ out_proj_b   # [B, L]

Strategy: expert-parallel layer 1 over the 8 NeuronCores of one trn2 chip
(13 expert slots/core, U padded 100->104). Each core streams its 13 [768,768]
expert matrices from HBM in bf16 (~15 MB/core — the memory roofline), computes
per-expert y_u = x @ W_u on the tensor engine, and folds it into a partial
z += mask[:,u] * y_u with one fused scalar_tensor_tensor per tile on DVE
(accumulated in bf16, k-columns permuted per-core into "local slice" order).

The cross-core reduction of z avoids the collectives stack entirely (the
cost of a single ReduceScatter exceeds the whole reduction here): a direct
SBUF-to-SBUF remote-DMA XOR mesh. Local slice k (96 k-columns) is the global
slice owned by the core at physical XOR-distance k; when the stream ends the
core fires 7 remote_dma_broadcast sends (relative dests, drid=0, dtpb=k, one
SWDGE trigger for all of them — partial triggers proved unreliable on HW)
that land in the peers' receive slots and bump a per-slice remote semaphore.
Each core sums its own slice plus the 7 received contributions as they
arrive (Pool-engine adds gated per-slot), applies tanh, and runs its 96-row
share of the output projection against all 100 users; the host sums the 8
[256, 2] partials.

The physical XOR topology (logical->physical TPB permutation) is discovered
once at runtime by a tiny probe kernel; a hardcoded trn2 fallback is used if
the probe fails.
"""
import numpy as np

B, S, H, U, L = 256, 512, 768, 100, 2
N_CORES = 8
UPC = 13            # expert slots per core (8*13 = 104 >= 100, zero-padded)
UPAD = N_CORES * UPC
NB = B // 128       # 2 batch tiles
NH = H // 128       # 6 contraction chunks
SLW = H // N_CORES  # 96-column slice owned by each core
# the send slices (1..7) stream in two PSUM-sized pieces; the own slice 0 is
# computed in a final mini-pass from the resident expert tiles, so the sends
# overlap it. Experts are loaded full-width once and stay resident in SBUF.
PIECES = ((1, 5), (5, 8))   # piece -> (first slice, end slice)
L2W = L * U         # 200 columns of the layer-2 moving operand

_RUNNER = None
_PERM = None


def _build_nc():
    import concourse.bacc as bacc
    import concourse.mybir as mybir
    import concourse.tile as tile
    from concourse.masks import make_identity

    f32 = mybir.dt.float32
    f32r = mybir.dt.float32r
    bf16 = mybir.dt.bfloat16

    nc = bacc.Bacc("TRN2", target_bir_lowering=False)

    xT = nc.dram_tensor("xT", [H, B], bf16, kind="ExternalInput")
    w = nc.dram_tensor("w", [UPC, H, H], bf16, kind="ExternalInput")
    mask = nc.dram_tensor("mask", [B, UPC], f32, kind="ExternalInput")
    maskT = nc.dram_tensor("maskT", [UPC, B], f32r, kind="ExternalInput")
    db = nc.dram_tensor("db", [UPC, H], f32r, kind="ExternalInput")
    woT = nc.dram_tensor("woT", [SLW, L2W], bf16, kind="ExternalInput")
    bo = nc.dram_tensor("bo", [1, L2W], f32, kind="ExternalInput")
    mrep = nc.dram_tensor("mrep", [B, L2W], f32, kind="ExternalInput")
    o = nc.dram_tensor("o", [B, L], f32, kind="ExternalOutput")

    rsems = [nc.alloc_semaphore(f"rsem{k}") for k in range(1, N_CORES)]
    lsem = nc.alloc_semaphore("lsem")
    gsem = nc.alloc_semaphore("gsem")
    dsem = nc.alloc_semaphore("dsem")
    waits_to_patch = []
    trig_holder = []

    with tile.TileContext(nc) as tc:
        with (
            tc.tile_pool(name="const", bufs=1) as cpool,
            tc.tile_pool(name="wpool", bufs=6) as wpool,
            tc.tile_pool(name="py", bufs=4, space="PSUM") as psum_y,
            tc.tile_pool(name="pmisc", bufs=2, space="PSUM") as psum_m,
            tc.tile_pool(name="pq", bufs=2, space="PSUM") as psum_q,
        ):
            # --- resident inputs (xT + first weights first: they gate the
            # stream; layer-2 inputs are deferred until the stream is going) ---
            xT_sb = cpool.tile([128, NH, B], bf16, tag="xT")
            nc.sync.dma_start(xT_sb[:], xT.rearrange("(p c) b -> p c b", p=128))
            wt0 = cpool.tile([128, NH, H], bf16, tag="w0")
            w_tiles = [wt0]
            nc.sync.dma_start(
                wt0[:], w[0].rearrange("(p c) k -> p c k", p=128))
            maskT_sb = cpool.tile([UPC, B], f32r, tag="maskT")
            nc.sync.dma_start(maskT_sb[:], maskT[:])
            db_sb = cpool.tile([UPC, H], f32r, tag="db")
            nc.sync.dma_start(db_sb[:], db[:])
            mask_sb = cpool.tile([128, NB, UPC], f32, tag="mask")
            nc.sync.dma_start(mask_sb[:], mask.rearrange("(nb p) u -> p nb u", p=128))
            ones = cpool.tile([1, 512], f32, tag="ones")
            nc.vector.memset(ones[:], 1.0)
            for u in range(1, UPC):
                wt = cpool.tile([128, NH, H], bf16, tag=f"w{u}")
                nc.sync.dma_start(
                    wt[:], w[u].rearrange("(p c) k -> p c k", p=128))
                w_tiles.append(wt)

            # p-state warmup: keep PE busy continuously from t~1us until the
            # first expert weights land, so the 2.4 GHz clock is reached
            # before the real stream starts (junk matmuls, never read)
            warm = psum_y.tile([128, 480], f32, tag="y")
            for cols in (480, 480, 384):
                nc.tensor.matmul(warm[:, :cols], ones[:, :128], ones[:, :cols],
                                 start=True, stop=True, skip_group_check=True)

            # z accumulator, k-columns grouped by local slice (bf16)
            z2_sb = cpool.tile([128, N_CORES, NB, SLW], bf16, tag="z2")
            # peer contributions land here: slot k-1 <- peer at XOR-distance k
            recv_sb = cpool.tile([128, N_CORES - 1, NB, SLW], bf16, tag="recv")

            # descriptor generation for all 7 sends runs up front on the
            # idle Pool engine (descgen reads no data); the trigger's wait is
            # patched after scheduling to the final STT's DVE engine tick,
            # which on the in-order DVE queue covers every z2 write.
            for k in range(1, N_CORES):
                nc.gpsimd.remote_dma_broadcast(
                    recv_sb[:, k - 1, :, :], z2_sb[:, k, :, :],
                    remote_sem=rsems[k - 1], local_sem=lsem,
                    rdests=[(0, k) if j == k else None
                            for j in range(N_CORES)],
                )


            # --- expert stream: z += mask[:,u] * (x @ W_u), piece by piece;
            # after a piece completes its slices are sent to their owners ---
            stream_stts = []
            for u in range(UPC):
                for pi, (s0, s1) in enumerate(PIECES):
                    pw = (s1 - s0) * SLW
                    w_sb = w_tiles[u][:, :, s0 * SLW:s1 * SLW]
                    for b in range(NB):
                        acc = psum_y.tile([128, 480], f32, tag="y")
                        for hc in range(NH):
                            nc.tensor.matmul(
                                acc[:, :pw],
                                xT_sb[:, hc, b * 128:(b + 1) * 128],
                                w_sb[:, hc, :],
                                start=(hc == 0), stop=(hc == NH - 1),
                            )
                        if pi == 0 and u == 0:
                            # layer-1 bias partial seeds z, both pieces
                            # (placed here so the tiny seed matmuls don't
                            # delay the stream start)
                            for (ss0, ss1) in ((0, 1),) + PIECES:
                                sw = (ss1 - ss0) * SLW
                                sacc = psum_y.tile([128, 480], f32, tag="y")
                                nc.tensor.matmul(
                                    sacc[:, :sw],
                                    maskT_sb[:, b * 128:(b + 1) * 128],
                                    db_sb[:, ss0 * SLW:ss1 * SLW],
                                    start=True, stop=True,
                                )
                                nc.scalar.copy(
                                    z2_sb[:, ss0:ss1, b, :],
                                    sacc[:, :sw].rearrange(
                                        "p (s k) -> p s k", k=SLW),
                                )
                        stream_stts.append(nc.vector.scalar_tensor_tensor(
                            z2_sb[:, s0:s1, b, :],
                            acc[:, :pw].rearrange("p (s k) -> p s k", k=SLW),
                            mask_sb[:, b, u:u + 1],
                            z2_sb[:, s0:s1, b, :],
                            mybir.AluOpType.mult, mybir.AluOpType.add,
                        ))
                # one trigger fires all 7 sends (the SWDGE trigger path
                # misbehaves on overlapping triggers); gsem orders the reduce
                # adds after the trigger in the scheduler's in-order Pool
                # queue. The dsem placeholder wait is rewritten after
                # scheduling to the final STT's DVE-tick condition.
                if u == UPC - 1 and pi == len(PIECES) - 1:
                    trig = nc.gpsimd.trigger_dma(count=None)._wait_ge(dsem, 0)
                    trig.then_inc(gsem)
                    trig_holder.append(trig)
                    # own-slice mini-pass (cols 0..96) overlaps the sends
                    for u in range(UPC):
                        for b in range(NB):
                            acc = psum_y.tile([128, 480], f32, tag="y")
                            for hc in range(NH):
                                nc.tensor.matmul(
                                    acc[:, :SLW],
                                    xT_sb[:, hc, b * 128:(b + 1) * 128],
                                    w_tiles[u][:, hc, 0:SLW],
                                    start=(hc == 0), stop=(hc == NH - 1),
                                )
                            nc.vector.scalar_tensor_tensor(
                                z2_sb[:, 0:1, b, :],
                                acc[:, :SLW].rearrange("p (s k) -> p s k", k=SLW),
                                mask_sb[:, b, u:u + 1],
                                z2_sb[:, 0:1, b, :],
                                mybir.AluOpType.mult, mybir.AluOpType.add,
                            )

            # --- reduce: own slice + 7 peer contributions (f32 accumulator).
            # The first 4 arrive with piece 0, the rest with piece 1; the
            # remote-sem waits are patched in after scheduling (the Tile
            # scheduler's single-core sim cannot see remote increments). ---
            ps_sb = cpool.tile([128, NB, SLW], f32, tag="ps")
            nc.gpsimd.memset(ps_sb[:], 0.0)
            for j in range(N_CORES - 1):
                ij = nc.gpsimd.tensor_add(
                    ps_sb[:], ps_sb[:], recv_sb[:, j, :, :]
                )._wait_ge(gsem, 1)
                waits_to_patch.append((ij, rsems[j], 2))
            nc.gpsimd.tensor_add(ps_sb[:], ps_sb[:], z2_sb[:, 0, :, :])

            # --- deferred layer-2 inputs ---
            woT_sb = cpool.tile([SLW, L2W], bf16, tag="woT")
            nc.sync.dma_start(woT_sb[:], woT[:])
            bo_sb = cpool.tile([1, L2W], f32, tag="bo")
            nc.sync.dma_start(bo_sb[:], bo[:])
            mrep_sb = cpool.tile([128, NB, L2W], f32, tag="mrep")
            nc.sync.dma_start(mrep_sb[:], mrep.rearrange("(nb p) l -> p nb l", p=128))
            ident = cpool.tile([128, 128], f32, tag="ident")
            make_identity(nc, ident[:])

            # --- tail: transpose + tanh the owned slice, tiny layer 2 ---
            o_sb = cpool.tile([128, NB, L], f32, tag="o")
            for b in range(NB):
                tp = psum_m.tile([SLW, 128], f32, tag="tp")
                nc.tensor.transpose(tp[:], ps_sb[:, b, :], ident[:])
                hT_sb = cpool.tile([SLW, 128], bf16, tag=f"hT{b}")
                nc.scalar.activation(
                    hT_sb[:], tp[:], mybir.ActivationFunctionType.Tanh)
                q = psum_q.tile([128, L2W], f32, tag="q")
                nc.tensor.matmul(q[:], ones[:, :128], bo_sb[:],
                                 start=True, stop=False)
                nc.tensor.matmul(q[:], hT_sb[:], woT_sb[:],
                                 start=False, stop=True)
                p2 = cpool.tile([128, L2W], f32, tag=f"p2{b}")
                nc.vector.tensor_mul(p2[:], q[:], mrep_sb[:, b, :])
                nc.vector.reduce_sum(
                    o_sb[:, b, :],
                    p2[:].rearrange("p (l u) -> p l u", u=U),
                    axis=mybir.AxisListType.X,
                )
            nc.sync.dma_start(o.rearrange("(nb p) l -> p nb l", p=128), o_sb[:])

    # Gate the trigger on the final STT's completion: find the DVE engine
    # tick the last STT updates and the cumulative count at that point, and
    # rewrite the trigger's dsem placeholder wait to it.
    import concourse.bass_isa as bass_isa
    insts = [i for blk in nc.m.functions[0].blocks for i in blk.instructions]
    last_stt = stream_stts[-1].ins
    upd = [u for u in last_stt.sync_info.on_update
           if u.ant_name and u.ant_name.startswith("DVE")]
    assert upd, "last STT carries no DVE tick update"
    tick_id, tick_name = upd[0].id, upd[0].ant_name
    count = 0
    for i in insts:
        si = getattr(i, "sync_info", None)
        if si is None:
            continue
        for u in si.on_update:
            if u.id == tick_id:
                count += u.update_value if u.update_value is not None else 1
        if i is last_stt:
            break
    for sw in trig_holder[0].ins.sync_info.on_wait:
        if sw.ant_name == "dsem":
            sw.id = tick_id
            sw.ant_name = tick_name
            sw.wait_value = count

    # Rewrite the adds' schedule-ordering gsem waits into the true per-slot
    # arrival waits (2 per send on that slice's remote sem).
    for inst, sem, val in waits_to_patch:
        for sw in inst.ins.sync_info.on_wait:
            if sw.ant_name == "gsem":
                sw.id = sem.num
                sw.ant_name = sem.name
                sw.wait_value = val
    nc.finalize()
    nc._rsems = {sem.name: sem for sem in rsems}
    return nc


def _sim_cost_model():
    """Cost model for TimelineSim that fills the stock model's no_exec gap for
    the remote-DMA mesh: remote_dma_broadcast preps are recorded at visit
    time, and the matching InstTriggerDma gets an extra DMA_ENGINES track
    carrying the transfer delays plus the remote-sem updates. Each update is
    applied to the LOCAL remote-sem — the single-core timeline proxy for a
    symmetric SPMD mesh (my send to a peer completes when the peer's
    symmetric send to me lands here)."""
    import concourse.mybir as mybir
    import concourse.bass_isa as bass_isa
    from concourse.cost_model import (
        Delay, DeviceAcquire, DeviceFree, InstructionCostModel,
        NonEngineDevice, SemUpdate,
    )
    from concourse.hw_specs import TRN2Spec

    SEQ = mybir.EngineType.Pool

    class RdmaLoopback(InstructionCostModel):
        rsems = None

        def __init__(self, hw_spec):
            super().__init__(hw_spec)
            self._pending = {}

        def visit(self, instruction, sim):
            tracks = self._state.visit(instruction, sim)
            if isinstance(instruction, bass_isa.InstRemoteDMABroadcastDescs):
                n_dests = len(instruction.dests)
                n_real = sum(1 for d in instruction.dests if d >= 0)
                bw = TRN2Spec.RDMA_D2D_BANDWIDTH_BYTES_PER_NS_PER_ENGINE * 16.0
                self._pending.setdefault(instruction.queue_num, []).append((
                    n_dests * instruction.free_dim_bytes * 128.0 / bw,
                    instruction.remote_sem_name,
                    n_real * (16 // max(n_dests, 1)),
                ))
            elif (isinstance(instruction, bass_isa.InstTriggerDma)
                  and self._pending.get(instruction.queue_num)
                  and self.rsems is not None):
                fire = self._pending.pop(instruction.queue_num)
                extra = [
                    DeviceAcquire(_engine_device(SEQ)),
                    Delay(1.0),
                    DeviceFree(_engine_device(SEQ)),
                    DeviceAcquire(NonEngineDevice.DMA_ENGINES),
                ]
                from concourse.bass import create_sync_update
                for t_ns, sem_name, inc in fire:
                    extra.append(Delay(t_ns))
                    extra.append(SemUpdate(
                        create_sync_update(self.rsems[sem_name], inc)))
                extra.append(Delay(TRN2Spec.RDMA_D2D_ACK_LATENCY_NS))
                extra.append(DeviceFree(NonEngineDevice.DMA_ENGINES))
                tracks = list(tracks) + [extra]
            return tracks

    def _engine_device(engine):
        # Device = tuple[EngineType, EngComponent] on the Python side
        from concourse.hw_specs import EngComponent
        return (engine, EngComponent.SEQ)

    return RdmaLoopback(TRN2Spec)


def sim_ns():
    """Cost-model timeline estimate for this kernel (ns)."""
    from concourse.timeline_sim import TimelineSim
    nc = _build_nc()
    cm = _sim_cost_model()
    cm.rsems = nc._rsems
    return TimelineSim(nc, cost_model=cm).simulate()


# ---------------------------------------------------------------------------
# Topology probe: seen[c][k] = logical id of the core at physical XOR
# distance k from core c. Row c is also the slice permutation for core c.
# ---------------------------------------------------------------------------

def _fallback_perm():
    # trn2 logical->physical TPB map: dies pair as P(c) = c ^ (2 if c&4)
    P = [c ^ 2 if c & 4 else c for c in range(N_CORES)]
    return np.array([[P[P[c] ^ k] for k in range(N_CORES)]
                     for c in range(N_CORES)], dtype=np.int64)


def _probe_nc():
    import concourse.bacc as bacc
    import concourse.mybir as mybir
    import concourse.tile as tile

    f32 = mybir.dt.float32
    W = 8
    nc = bacc.Bacc("TRN2", target_bir_lowering=False)
    tag = nc.dram_tensor("tag", [128, W], f32, kind="ExternalInput")
    seen = nc.dram_tensor("seen", [1, N_CORES * W], f32, kind="ExternalOutput")
    rsem = nc.alloc_semaphore("rsem")
    lsem = nc.alloc_semaphore("lsem")
    with tile.TileContext(nc) as tc:
        with tc.tile_pool(name="p", bufs=1) as pool:
            tag_sb = pool.tile([128, W], f32, tag="tag")
            nc.sync.dma_start(tag_sb[:], tag[:])
            recv_sb = pool.tile([128, N_CORES, W], f32, tag="recv")
            nc.vector.memset(recv_sb[:], -1.0)
            for k in range(N_CORES):
                nc.gpsimd.remote_dma_broadcast(
                    recv_sb[:, k, :], tag_sb[:],
                    remote_sem=rsem, local_sem=lsem,
                    rdests=[(0, k) if j == k else None for j in range(N_CORES)],
                )
            nc.gpsimd.trigger_dma(count=None)
            out_dma = nc.sync.dma_start(
                seen.rearrange("o (k w) -> o k w", k=N_CORES),
                recv_sb[0:1, :, :],
            )._wait_ge(rsem, 0)
    for sw in out_dma.ins.sync_info.on_wait:
        if sw.ant_name == "rsem":
            sw.wait_value = 2 * N_CORES
    nc.finalize()
    return nc


def _discover_perm():
    global _PERM
    if _PERM is not None:
        return _PERM
    try:
        runner = _SpmdRunner(_probe_nc(), N_CORES)
        in_maps = [{"tag": np.full((128, 8), float(c), np.float32)}
                   for c in range(N_CORES)]
        results = runner.run(in_maps)
        perm = np.zeros((N_CORES, N_CORES), dtype=np.int64)
        for c in range(N_CORES):
            row = results[c]["seen"].reshape(N_CORES, 8)[:, 0]
            perm[c] = row.astype(np.int64)
        valid = all(sorted(perm[c].tolist()) == list(range(N_CORES))
                    and perm[c][0] == c for c in range(N_CORES))
        _PERM = perm if valid else _fallback_perm()
    except Exception:
        _PERM = _fallback_perm()
    return _PERM


class _SpmdRunner:
    """Cached PJRT SPMD runner (mirrors concourse.bass2jax.run_bass_via_pjrt,
    but keeps the jitted callable alive so repeat calls don't re-trace)."""

    def __init__(self, nc, n_cores):
        import jax
        import concourse.mybir as mybir
        from concourse.bass2jax import (
            _bass_exec_p, install_neuronx_cc_hook, partition_id_tensor,
        )
        from jax.sharding import Mesh, PartitionSpec, NamedSharding
        try:
            from jax.experimental.shard_map import shard_map
        except ImportError:
            from jax.shard_map import shard_map

        install_neuronx_cc_hook()
        self.jax = jax
        self.nc = nc
        self.n_cores = n_cores

        in_names, out_names, out_avals, zero_outs = [], [], [], []
        partition_name = nc.partition_id_tensor.name if nc.partition_id_tensor else None
        dbg_name = None
        if nc.dbg_addr is not None:
            assert not nc.dbg_callbacks
            dbg_name = nc.dbg_addr.name
        for alloc in nc.m.functions[0].allocations:
            if not isinstance(alloc, mybir.MemoryLocationSet):
                continue
            name = alloc.memorylocations[0].name
            if alloc.kind == "ExternalInput":
                if name not in (partition_name, dbg_name):
                    in_names.append(name)
            elif alloc.kind == "ExternalOutput":
                out_names.append(name)
                shape = tuple(alloc.tensor_shape)
                dtype = mybir.dt.np(alloc.dtype)
                out_avals.append(jax.core.ShapedArray(shape, dtype))
                zero_outs.append(np.zeros(shape, dtype))

        self.in_names = list(in_names)
        self.out_names = list(out_names)
        self.zero_outs = zero_outs

        n_params = len(in_names)
        bound_names = list(in_names) + list(out_names)
        if dbg_name is not None:
            bound_names.append(dbg_name)
        if partition_name is not None:
            bound_names.append(partition_name)

        def _body(*args):
            operands = list(args)
            if dbg_name is not None:
                operands.append(jax.numpy.zeros((1, 2), jax.numpy.uint32))
            if partition_name is not None:
                operands.append(partition_id_tensor())
            outs = _bass_exec_p.bind(
                *operands,
                out_avals=tuple(out_avals),
                in_names=tuple(bound_names),
                out_names=tuple(self.out_names),
                lowering_input_output_aliases=(),
                sim_require_finite=True,
                sim_require_nnan=True,
                nc=nc,
            )
            return tuple(outs)

        devices = jax.devices()[:n_cores]
        assert len(devices) == n_cores, f"need {n_cores} cores, have {len(devices)}"
        self.mesh = Mesh(np.asarray(devices), ("core",))
        self.spec = PartitionSpec("core")
        self.sharding = NamedSharding(self.mesh, self.spec)
        n_args = n_params + len(out_names)
        self._jit = jax.jit(
            shard_map(
                _body,
                mesh=self.mesh,
                in_specs=(self.spec,) * n_args,
                out_specs=(self.spec,) * len(out_names),
                check_rep=False,
            ),
            keep_unused=True,
        )

    def put(self, in_maps):
        args = []
        for name in self.in_names:
            arrs = [np.asarray(in_maps[c][name]) for c in range(self.n_cores)]
            args.append(np.concatenate(arrs, axis=0))
        for z in self.zero_outs:
            args.append(np.concatenate([z] * self.n_cores, axis=0))
        return [self.jax.device_put(a, self.sharding) for a in args]

    def run_device(self, device_args):
        return self._jit(*device_args)

    def run(self, in_maps):
        outs = self._jit(*self.put(in_maps))
        np_outs = [np.asarray(o) for o in outs]
        results = []
        for c in range(self.n_cores):
            d = {}
            for i, name in enumerate(self.out_names):
                full = np_outs[i]
                per = full.shape[0] // self.n_cores
                d[name] = full[c * per:(c + 1) * per]
            results.append(d)
        return results


def _get_runner():
    global _RUNNER
    if _RUNNER is None:
        _RUNNER = _SpmdRunner(_build_nc(), N_CORES)
    return _RUNNER


def _prep_in_maps(hidden_states, user_mask, dense_W, dense_b, out_proj_W, out_proj_b):
    import ml_dtypes
    bf16 = ml_dtypes.bfloat16

    perm = _discover_perm()

    x = np.ascontiguousarray(hidden_states[:, 0, :], dtype=np.float32)   # [B, H]
    xT = np.ascontiguousarray(x.T).astype(bf16)                          # [H, B]

    # pad experts to 104
    pad = UPAD - U
    mask_p = np.concatenate([user_mask, np.zeros((B, pad), np.float32)], axis=1)
    w_p = np.concatenate([dense_W, np.zeros((pad, H, H), np.float32)], axis=0)
    db_p = np.concatenate([dense_b, np.zeros((pad, H), np.float32)], axis=0)

    # layer 2: every core gets its own 96-row slice of the full user bank,
    # the full mask (repeated for both logits), and bias/8 (host sums cores)
    woT_full = np.ascontiguousarray(
        out_proj_W.transpose(1, 2, 0).reshape(H, L2W)).astype(bf16)      # [H,(l,u)]
    bo8 = np.ascontiguousarray(out_proj_b.T.reshape(1, L2W)) / N_CORES   # [1,(l,u)]
    mrep = np.ascontiguousarray(
        np.concatenate([user_mask, user_mask], axis=1), dtype=np.float32)

    in_maps = []
    for c in range(N_CORES):
        sl = slice(c * UPC, (c + 1) * UPC)
        m = np.ascontiguousarray(mask_p[:, sl])                 # [B, UPC]
        # local slice k holds the global slice owned by the peer at physical
        # XOR distance k (perm[c][k]); k-columns of W/db permuted to match
        cols = np.concatenate(
            [np.arange(g * SLW, (g + 1) * SLW) for g in perm[c]])
        in_maps.append({
            "xT": xT,
            "w": np.ascontiguousarray(w_p[sl][:, :, cols]).astype(bf16),
            "mask": m,
            "maskT": np.ascontiguousarray(m.T),
            "db": np.ascontiguousarray(db_p[sl][:, cols]),
            "woT": np.ascontiguousarray(woT_full[c * SLW:(c + 1) * SLW]),
            "bo": bo8.astype(np.float32),
            "mrep": mrep,
        })
    return in_maps


def kernel(hidden_states, user_mask, dense_W, dense_b, out_proj_W, out_proj_b):
    hidden_states = np.asarray(hidden_states, dtype=np.float32)
    user_mask = np.asarray(user_mask, dtype=np.float32)
    dense_W = np.asarray(dense_W, dtype=np.float32)
    dense_b = np.asarray(dense_b, dtype=np.float32)
    out_proj_W = np.asarray(out_proj_W, dtype=np.float32)
    out_proj_b = np.asarray(out_proj_b, dtype=np.float32)

    runner = _get_runner()
    in_maps = _prep_in_maps(hidden_states, user_mask, dense_W, dense_b,
                            out_proj_W, out_proj_b)
    results = runner.run(in_maps)
    out = np.zeros((B, L), np.float32)
    for c in range(N_CORES):
        out += results[c]["o"]
    return np.ascontiguousarray(out, dtype=np.float32)
